# revision 61
# baseline (speedup 1.0000x reference)
"""Neighborhood attention (7x7) Trainium2 Bass kernel.

Sharding: 8 cores = 4 batches x 2 row-halves (32 rows each).
Per core: project q/k/v from a zero-padded 40-row slice, banded attention
via dense Gram matmuls ([key_px, query_px] orientation), masked exp,
Z and AV accumulated in PSUM region tiles, out-projection.

V2 speedups over the original:
  - bf16 / float32r matmuls everywhere (1 cyc/row vs 4 for fp32)
  - x, weights, q, k shipped/stored as bf16 (halves DMA and SBUF traffic)
  - bq folded into the q projection bias (per-partition add) instead of
    the exp(u) multiplicative machinery; Z is a ones-column matmul
  - Z accumulates in PSUM across kblocks (no DVE adds)
  - exp writes bf16, mask-mul runs in DVE 4x mode, masks shipped as bf16
  - software-pipelined emission: exp/mask(kb-1) -> S(kb) -> Z/AV(kb-1),
    out-projection of region r interleaved right after its g=1 finalize
  - input DMAs split across the SP and Activation HWDGE queues
"""
import sys
import numpy as np
from contextlib import ExitStack

sys.path.insert(0, "/opt/trn_rl_repo")

import concourse.bass as bass
import concourse.bacc as bacc
import concourse.mybir as mybir
import concourse.tile as tile
from concourse.bass_utils import run_bass_kernel_spmd

import ml_dtypes

DIM, HEADS, HD = 256, 8, 32
SCALE = HD ** -0.5
B, H, W = 4, 64, 64
KVR = 38          # kv rows per core (zero padded)
NKB = KVR // 2    # 19 kblocks of 2 rows (128 px)
NPX = KVR * 64    # 2432 kv pixels
QOFF = 3 * 64     # own-query offset inside kv pixels
NQ = 2048         # own query pixels (32 rows)
F32 = mybir.dt.float32
F32R = mybir.dt.float32r
BF16 = mybir.dt.bfloat16


def _core_geom(core):
    b, half = core // 2, core % 2
    return b, half * 32  # batch, R0


def _qcol0(kb):
    return 64 * min(max(2 * kb - 6, 0), 24)


def _win(kb):
    """(qcol0, off, w): 512-window start, valid-span offset and width."""
    lo_r = min(max(2 * kb - 6, 0), 24)
    v0 = max(2 * kb - 6, 0)
    v1 = min(2 * kb + 2, 32)
    return lo_r * 64, (v0 - lo_r) * 64, (v1 - v0) * 64


def _contribs():
    """region -> list of kb contributing"""
    out = []
    for r in range(4):
        kbs = []
        for kb in range(NKB):
            qc0, off, w = _win(kb)
            if qc0 + off < 512 * r + 512 and qc0 + off + w > 512 * r:
                kbs.append(kb)
        out.append(kbs)
    return out


def _build_mask(R0, kb):
    """[128 kpx, 512 qwin] in {0,1}, window rows = R0 + lo_rel .. +8"""
    krow = R0 - 3 + 2 * kb
    lo_r = R0 + _qcol0(kb) // 64
    kp = np.arange(128)
    rk = krow + kp // 64
    ck = kp % 64
    qc = np.arange(512)
    rq = lo_r + qc // 64
    cq = qc % 64
    ok_k = ((rk >= 0) & (rk < H))[:, None]
    band = (np.abs(rk[:, None] - rq[None, :]) <= 3) & \
           (np.abs(ck[:, None] - cq[None, :]) <= 3)
    return (ok_k & band).astype(np.float32)


def _build_program():
    nc = bacc.Bacc(trn_type="TRN2", target_bir_lowering=False, debug=False,
                   num_devices=8)
    d = {}
    d["x"] = nc.dram_tensor("x", [DIM, NPX], BF16, kind="ExternalInput")
    for w in ["wqT", "wkT", "wvT", "woT"]:
        d[w] = nc.dram_tensor(w, [DIM, DIM], BF16, kind="ExternalInput")
    d["masks"] = nc.dram_tensor("masks", [128, NKB * 512], BF16,
                                kind="ExternalInput")
    d["repmat"] = nc.dram_tensor("repmat", [128, 128], BF16,
                                 kind="ExternalInput")
    d["bq2"] = nc.dram_tensor("bq2", [128, 2], F32, kind="ExternalInput")
    d["bo2"] = nc.dram_tensor("bo2", [128, 2], F32, kind="ExternalInput")
    y = nc.dram_tensor("y", [DIM, NQ], F32, kind="ExternalOutput")

    contribs = _contribs()
    first_kb = [min(k) for k in contribs]
    last_kb = [max(k) for k in contribs]

    with ExitStack() as ctx:
        tc = ctx.enter_context(tile.TileContext(nc))
        cp = ctx.enter_context(tc.tile_pool(name="const", bufs=1))
        sp = ctx.enter_context(tc.tile_pool(name="spsum", bufs=2, space="PSUM"))
        avp = ctx.enter_context(tc.tile_pool(name="avpsum", bufs=2, space="PSUM"))
        zp = ctx.enter_context(tc.tile_pool(name="zpsum", bufs=2, space="PSUM"))
        wp = ctx.enter_context(tc.tile_pool(name="work", bufs=4))
        otp = ctx.enter_context(tc.tile_pool(name="otp", bufs=6))

        # ---- load constants / inputs (split across SP + ACT hwdge queues) ----
        x_sb = cp.tile([128, 2 * NPX], BF16)
        XH = 1280
        w_sb = {}
        for w in ["wqT", "wkT", "wvT", "woT"]:
            w_sb[w] = cp.tile([128, 512], BF16, tag=w, name=w)
        bq2_sb = cp.tile([128, 2], F32)
        nc.sync.dma_start(bq2_sb[:], d["bq2"][:])
        for kt in range(2):
            nc.scalar.dma_start(w_sb["wqT"][:, 256 * kt:256 * (kt + 1)],
                                d["wqT"][128 * kt:128 * (kt + 1), :])
        first = True
        for h0 in range(0, NPX, XH):
            hw_ = min(XH, NPX - h0)
            nc.sync.dma_start(x_sb[:, h0:h0 + hw_],
                              d["x"][0:128, h0:h0 + hw_])
            nc.scalar.dma_start(x_sb[:, NPX + h0:NPX + h0 + hw_],
                                d["x"][128:256, h0:h0 + hw_])
            if first:
                for kt in range(2):
                    nc.sync.dma_start(w_sb["wkT"][:, 256 * kt:256 * (kt + 1)],
                                      d["wkT"][128 * kt:128 * (kt + 1), :])
                first = False
        for w in ["wvT", "woT"]:
            for kt in range(2):
                nc.sync.dma_start(w_sb[w][:, 256 * kt:256 * (kt + 1)],
                                  d[w][128 * kt:128 * (kt + 1), :])
        bo2_sb = cp.tile([128, 2], F32)
        nc.scalar.dma_start(bo2_sb[:], d["bo2"][:])
        repmat_sb = cp.tile([128, 128], BF16)
        nc.scalar.dma_start(repmat_sb[:], d["repmat"][:])
        masks_sb = cp.tile([128, NKB * 512], BF16)
        nc.scalar.dma_start(masks_sb[:], d["masks"][:])

        ones_sb = cp.tile([128, 1], BF16)
        nc.vector.memset(ones_sb[:], 1.0)
        zrow_sb = cp.tile([1, 512], BF16)
        nc.vector.memset(zrow_sb[:], 0.0)
        onerow_sb = cp.tile([1, 512], BF16)
        nc.vector.memset(onerow_sb[:], 1.0)
        epsrow_sb = cp.tile([1, 512], BF16)
        nc.vector.memset(epsrow_sb[:], 1e-30)

        q_sb = [cp.tile([128, NQ], BF16, tag=f"q{m}", name=f"q{m}")
                for m in range(2)]
        k_sb = [cp.tile([128, NPX], BF16, tag=f"k{m}", name=f"k{m}") for m in range(2)]
        avn_sb = cp.tile([128, 2 * NQ], BF16)
        v2_sb = cp.tile([128, NKB * 256], BF16)

        # ---- projections (q, k natural; v form-B) ----
        ppools = [(sp, "s"), (avp, "av"), (zp, "z")]
        pidx = 0

        def _ptile(shape, name):
            nonlocal pidx
            p, tag = ppools[pidx % 3]
            pidx += 1
            return p.tile(shape, F32, tag=tag, name=name)

        kchunks = []
        c = 0
        while c < NPX:
            w_ = min(512, NPX - c)
            kchunks.append((c, w_))
            c += w_

        def emit_vproj():
            for t in range(NKB):
                ps = _ptile([128, 256], f"psv{t}")
                for kt in range(2):
                    lhsT = x_sb[:, NPX * kt + 128 * t:NPX * kt + 128 * t + 128]
                    nc.tensor.matmul(ps[:],
                                     lhsT, w_sb["wvT"][:, 256 * kt:256 * kt + 256],
                                     start=(kt == 0), stop=(kt == 1))
                if t % 2 == 0:
                    nc.scalar.copy(v2_sb[:, 256 * t:256 * t + 256], ps[:])
                else:
                    nc.vector.tensor_copy(v2_sb[:, 256 * t:256 * t + 256], ps[:])

        for m in range(2):
            if m == 1:
                emit_vproj()
            for n in range(NQ // 512):
                ps = _ptile([128, 512], f"psq{m}_{n}")
                for kt in range(2):
                    nc.tensor.matmul(
                        ps[:],
                        w_sb["wqT"][:, 256 * kt + 128 * m:256 * kt + 128 * m + 128],
                        x_sb[:, NPX * kt + QOFF + 512 * n:
                             NPX * kt + QOFF + 512 * n + 512],
                        start=(kt == 0), stop=(kt == 1))
                nc.vector.tensor_scalar_add(q_sb[m][:, 512 * n:512 * n + 512],
                                            ps[:], bq2_sb[:, m:m + 1])
            for i, (c0, w_) in enumerate(kchunks):
                ps = _ptile([128, w_], f"psk{m}_{i}")
                for kt in range(2):
                    nc.tensor.matmul(
                        ps[:],
                        w_sb["wkT"][:, 256 * kt + 128 * m:256 * kt + 128 * m + 128],
                        x_sb[:, NPX * kt + c0:NPX * kt + c0 + w_],
                        start=(kt == 0), stop=(kt == 1))
                if i % 2 == 0:
                    nc.scalar.copy(k_sb[m][:, c0:c0 + w_], ps[:])
                else:
                    nc.vector.tensor_copy(k_sb[m][:, c0:c0 + w_], ps[:])

        # ---- attention: pipelined over (g, kb) ----
        msk = masks_sb[:]
        av_tiles = {}
        z_tiles = {}
        fin_done = {}
        pending_fin = []

        def emit_S(g, kb):
            qc0, off, w = _win(kb)
            halves = []
            for h2 in range(2):
                spsum = sp.tile([128, 1024], F32, tag="s", name=f"s{g}_{kb}_{h2}")
                for j in range(2):
                    hh = 2 * h2 + j
                    nc.tensor.matmul(
                        spsum[:, 512 * j:512 * j + w],
                        k_sb[g][32 * hh:32 * hh + 32, 128 * kb:128 * kb + 128],
                        q_sb[g][32 * hh:32 * hh + 32, qc0 + off:qc0 + off + w],
                        start=True, stop=True, tile_position=(32 * hh, 0))
                halves.append(spsum)
            return halves

        def emit_expmask(g, kb, halves):
            qc0, off, w = _win(kb)
            msks = []
            for h2 in range(2):
                exp_t = wp.tile([128, 2 * w], BF16, tag=f"exp{h2}")
                ph = halves[h2][:]
                pslc = bass.AP(ph.tensor, ph.offset,
                               [[ph.ap[0][0], 128], [512, 2], [1, w]])
                nc.scalar.activation(exp_t[:], pslc,
                                     mybir.ActivationFunctionType.Exp)
                msk_t = wp.tile([128, 2 * w], BF16, tag=f"msk{h2}")
                mask_bcast = bass.AP(msk.tensor, msk.offset + 512 * kb + off,
                                     [[NKB * 512, 128], [0, 2], [1, w]])
                nc.vector.tensor_mul(msk_t[:], exp_t[:], mask_bcast)
                msks.append(msk_t)
            return msks

        pending_op = []

        def emit_oproj(r, f0=0, fw=512):
            pending_op.append((r, f0, fw))

        def flush_oproj():
            for (r, f0, fw) in pending_op:
                _do_oproj(r, f0, fw)
            pending_op.clear()

        def _do_oproj(r, f0=0, fw=512):
            ps = sp.tile([128, 1024], F32, tag="s", name=f"psop{r}_{f0}")
            for m in range(2):
                for g in range(2):
                    nc.tensor.matmul(
                        ps[:, 512 * m:512 * m + fw],
                        w_sb["woT"][:, 256 * g + 128 * m:256 * g + 128 * m + 128],
                        avn_sb[:, NQ * g + 512 * r + f0:
                               NQ * g + 512 * r + f0 + fw],
                        start=(g == 0), stop=(g == 1))
            ot = otp.tile([128, 2 * fw], F32, tag="ot", name=f"ot{r}_{f0}")
            for m in range(2):
                nc.vector.tensor_scalar_add(ot[:, fw * m:fw * m + fw],
                                            ps[:, 512 * m:512 * m + fw],
                                            bo2_sb[:, m:m + 1])
            oslc = ot[:]
            in_ap = bass.AP(oslc.tensor, oslc.offset,
                            [[oslc.ap[0][0], 128], [fw, 2], [1, fw]])
            ypitch = y[:].ap[0][0]
            out_ap = bass.AP(y[:].tensor, y[:].offset + 512 * r + f0,
                             [[ypitch, 128], [128 * ypitch, 2], [1, fw]])
            nc.sync.dma_start(out_ap, in_ap)

        def init_region(g, r):
            avt = avp.tile([128, 512], F32, tag="av", name=f"av{g}_{r}")
            av_tiles[(g, r)] = avt
            zt = zp.tile([128, 512], F32, tag="z", name=f"z{g}_{r}")
            z_tiles[(g, r)] = zt
            nc.vector.memset(avt[:], 0.0)
            nc.vector.memset(zt[:], 1e-30)

        def emit_zav(g, kb, msks):
            qc0, off, w = _win(kb)
            v0 = qc0 + off
            for r in range(4):
                if kb not in contribs[r]:
                    continue
                if kb == first_kb[r] and (g, r) not in av_tiles:
                    init_region(g, r)
                avt = av_tiles[(g, r)]
                zt = z_tiles[(g, r)]
                c0 = max(512 * r, v0)
                c1 = min(512 * r + 512, v0 + w)
                last = (kb == last_kb[r])
                # Region 3: columns left of the next kb's span are FINAL after
                # this kb. Close their accumulation group (stop=True) so the
                # finalize chain for most of the region runs before the last
                # kb, leaving only a short stub at the program end.
                c0r, c1r = c0 - 512 * r, c1 - 512 * r
                if r == 3 and kb < last_kb[r]:
                    nv0 = max(0, _win(kb + 1)[0] + _win(kb + 1)[1] - 512 * r)
                    fb = min(max(nv0, c0r), c1r)
                else:
                    fb = c1r if last else c0r
                segs = []
                if fb > c0r:
                    segs.append((c0r, fb, True))
                if c1r > fb:
                    segs.append((fb, c1r, False))
                for hh in range(4):
                    mt = msks[hh // 2]
                    for (s0, s1, st) in segs:
                        mslc = mt[:, w * (hh % 2) + s0 + 512 * r - v0:
                                  w * (hh % 2) + s1 + 512 * r - v0]
                        nc.tensor.matmul(
                            zt[32 * hh:32 * hh + 1, s0:s1],
                            ones_sb[:, 0:1], mslc,
                            start=False, stop=(st and hh == 3),
                            tile_position=(0, 32 * hh))
                        nc.tensor.matmul(
                            avt[32 * hh:32 * hh + 32, s0:s1],
                            v2_sb[:, 256 * kb + 32 * (4 * g + hh):
                                  256 * kb + 32 * (4 * g + hh) + 32],
                            mslc,
                            start=False, stop=(st and hh == 3),
                            tile_position=(0, 32 * hh))

                def _finalize(f0, fw):
                    zr_t = wp.tile([128, fw], BF16, tag="zr", name="zr_t")
                    with nc.allow_low_precision("bf16 1/Z feeds bf16 mm"):
                        nc.vector.reciprocal(zr_t[:], zt[:, f0:f0 + fw])
                    zrp = sp.tile([128, fw], F32, tag="s",
                                  name=f"zrp{g}_{r}_{f0}")
                    nc.tensor.matmul(zrp[:], repmat_sb[:], zr_t[:],
                                     start=True, stop=True)
                    zrep_t = wp.tile([128, fw], F32, tag="zrep", name="zrep_t")
                    nc.scalar.copy(zrep_t[:], zrp[:])
                    nc.vector.tensor_mul(
                        avn_sb[:, NQ * g + 512 * r + f0:
                               NQ * g + 512 * r + f0 + fw],
                        avt[:, f0:f0 + fw], zrep_t[:])
                    if g == 1:
                        emit_oproj(r, f0, fw)

                if r == 3 and g == 1:
                    done0 = fin_done.get((g, r), 0)
                    tgt = done0
                    if last:
                        tgt = 512
                    elif fb >= 384:
                        tgt = 384
                    if tgt > done0:
                        _finalize(done0, tgt - done0)
                        fin_done[(g, r)] = tgt
                elif r == 3 and last:
                    # defer g0-r3 finalize into early g1 (pool deadline: g1-kb4)
                    pending_fin.append((g, r, _finalize))
                    return
                elif last:
                    _finalize(0, 512)
                if last:
                    del av_tiles[(g, r)]
                    del z_tiles[(g, r)]

        prev = None
        for g in range(2):
            for kb in range(NKB):
                if prev is not None:
                    pm = emit_expmask(*prev)
                halves = emit_S(g, kb)
                flush_oproj()
                if g == 1 and kb == 2 and pending_fin:
                    for (pg, pr, fin) in pending_fin:
                        fin(0, 512)
                        del av_tiles[(pg, pr)]
                        del z_tiles[(pg, pr)]
                    pending_fin.clear()
                if prev is not None:
                    emit_zav(prev[0], prev[1], pm)
                prev = (g, kb, halves)
        pm = emit_expmask(*prev)
        flush_oproj()
        emit_zav(prev[0], prev[1], pm)
        flush_oproj()

    nc.compile()
    return nc


def _host_inputs(inputs):
    query = np.ascontiguousarray(inputs["query"], np.float32)
    wq = np.asarray(inputs["wq"], np.float32)
    bq = np.asarray(inputs["bq"], np.float32)
    wk = np.asarray(inputs["wk"], np.float32)
    wv = np.asarray(inputs["wv"], np.float32)
    bv = np.asarray(inputs["bv"], np.float32)
    wo = np.asarray(inputs["wo"], np.float32)
    bo = np.asarray(inputs["bo"], np.float32)

    bf = ml_dtypes.bfloat16
    wqT = np.ascontiguousarray((wq * SCALE).T).astype(bf)
    wkT = np.ascontiguousarray(wk.T).astype(bf)
    wvT = np.ascontiguousarray(wv.T).astype(bf)
    woT = np.ascontiguousarray(wo.T).astype(bf)
    bq2 = np.ascontiguousarray((SCALE * bq).reshape(2, 128).T)
    bo2v = (wo @ bv + bo).astype(np.float32)
    bo2 = np.ascontiguousarray(bo2v.reshape(2, 128).T)  # [128, 2] col m
    repmat = np.zeros((128, 128), np.float32)
    for pout in range(128):
        repmat[32 * (pout // 32), pout] = 1.0
    repmat = repmat.astype(bf)

    in_maps = []
    for core in range(8):
        b, R0 = _core_geom(core)
        xpad = np.zeros((DIM, KVR, 64), np.float32)
        lo, hi = R0 - 3, R0 + 35
        slo, shi = max(lo, 0), min(hi, H)
        xpad[:, slo - lo:shi - lo, :] = query[b][:, slo:shi, :]
        masks = np.concatenate([_build_mask(R0, kb) for kb in range(NKB)],
                               axis=1)
        in_maps.append({
            "x": np.ascontiguousarray(xpad.reshape(DIM, NPX)).astype(bf),
            "wqT": wqT, "wkT": wkT, "wvT": wvT, "woT": woT,
            "masks": np.ascontiguousarray(masks.astype(bf)),
            "repmat": repmat, "bq2": bq2, "bo2": bo2,
        })
    return in_maps


_nc_cache = None


def kernel(**inputs):
    global _nc_cache
    in_maps = _host_inputs(inputs)
    if _nc_cache is None:
        _nc_cache = _build_program()
    res = run_bass_kernel_spmd(_nc_cache, in_maps, core_ids=list(range(8)))
    out = np.zeros((B, DIM, H, W), np.float32)
    for core in range(8):
        b, R0 = _core_geom(core)
        out[b][:, R0:R0 + 32, :] = res.results[core]["y"].reshape(DIM, 32, 64)
    return out
